# revision 57
# baseline (speedup 1.0000x reference)
"""Multi-head attention (B=2, S=2048, D=1024, H=16) on 8 TRN2 NeuronCores.

Sharding: core c handles batch b = c//4 and heads [4*(c%4), 4*(c%4)+4) —
tensor-parallel over heads x data-parallel over batch.  Each core computes a
partial output projection (its heads' contribution); the host sums the 4
partials per batch and adds b_out.

On-device layout (per core; bf16 matmul operands, fp32 PSUM/softmax math):
  - qk projection computed transposed: qkT [512, S], row chunks
    [q_h0|q_h1, k_h0|k_h1, q_h2|q_h3, k_h2|k_h3].  Input DMAs are ordered
    (Wqk m-block 0, yT kc-slices for the first sb block, rest) so the first
    matmul starts ~5us after the engine preamble and the PE never starves.
  - scores computed transposed: expT[sk, sq] = exp(scale * kT.T @ qT); the
    softmax denominator comes free from a ones-column appended to v in the
    attn@v matmul (out row 64 = sum over sk of expT).  No max-subtraction:
    scores*scale is ~N(0,1) so fp32 exp cannot overflow.
  - exp is split across BOTH drain engines, alternating per (sub, mj) unit:
    ACT runs native Exp; DVE runs a one-op Schraudolph exp (bf16 bits =
    round(x*128*log2e + 127*128-7.05) via tensor_scalar with an int16
    bitcast output; ~1.8% RMS on those chunks, ~0.9% end to end).  With the
    ~17us/block of exp work split 8/8, neither engine's queue ever gates
    the score-PSUM ring, which is what kept tripping HAM re-throttles
    (PE at 1.2GHz) in earlier versions.
  - attn@v: valuesT_unnorm [65, sq] = v_aug.T @ expT accumulated over sk
    chunks in reversed order (one wait, then back-to-back so LDWEIGHTS
    hides), deferred one block so the exps are provably done.
  - normalize: vals[65] staged to SBUF (copy split ACT/DVE per sub), raw
    denominator row bounced through DRAM with a stride-0 partition
    broadcast, reciprocal_approx_fast at base partition 0 (the approx op
    misbehaves at nonzero base partitions), multiply on the otherwise-idle
    GPSIMD engine.  The v-bias is linear through the output projection and
    is added on the host instead.  In the epilogue the broadcast uses a
    K=1 ones matmul into a freed score-PSUM slot instead of the DRAM
    bounce (shorter critical path), and the two muls split DVE/GPSIMD.
  - output projection out_partial = valuesT.T @ WoutT: groups are queued
    when a sq block's four normalize chains have been issued and drained
    one-per-mj from mj 4 of later blocks, so every out matmul's vT inputs
    are always a block old and the PE stream never breaks.  PSUM staging
    copies go to ACT (it has slack; DVE is the busier drain engine).
"""
import sys

sys.path.insert(0, "/opt/trn_rl_repo")

import numpy as np

B, S, D = 2, 2048, 1024
H, Hd = 16, 64
P = 128
NKC = D // P      # 8 contraction chunks for the projections
NSC = S // P      # 16 sequence chunks of 128
SQB = 512         # sq block size
NSQB = S // SQB   # 4

_CACHE = {}


def _build_nc():
    import concourse.mybir as mybir
    import concourse.tile as tile
    from concourse import bacc

    f32 = mybir.dt.float32
    f32r = mybir.dt.float32r
    bf16 = mybir.dt.bfloat16
    i16 = mybir.dt.int16
    AF = mybir.ActivationFunctionType
    Alu = mybir.AluOpType

    # Schraudolph exp on DVE: bf16 bits = round(x*scale*SCH_A + SCH_B);
    # bitcast int16 write.  RMS rel err 1.8% on the tiles it handles;
    # end-to-end (4 of 16 sk-chunks) adds ~0.2% to the final output.
    LOG2E = float(np.log2(np.e))
    SCH_A = 128.0 * LOG2E
    SCH_B = 127.0 * 128.0 - 7.05


    nc = bacc.Bacc(None, target_bir_lowering=False, debug=False)

    yT_d = nc.dram_tensor("yT", [D, S], bf16, kind="ExternalInput")[:]
    Wqk_d = nc.dram_tensor("WqkT", [4, D, P], bf16, kind="ExternalInput")[:]
    bqk_d = nc.dram_tensor("bqk", [P, 4], f32, kind="ExternalInput")[:]
    Wv_d = nc.dram_tensor("WvT", [D, 256], bf16, kind="ExternalInput")[:]
    Wout_d = nc.dram_tensor("WoutT", [256, D], bf16, kind="ExternalInput")[:]
    out_d = nc.dram_tensor("out", [S, D], f32, kind="ExternalOutput")[:]

    with tile.TileContext(nc) as tc:
        with (
            tc.tile_pool(name="const", bufs=1) as const,
            tc.tile_pool(name="persist", bufs=1) as persist,
        ):
            Wout_sb = const.tile([P, 2, D], bf16)
            bqk_sb = const.tile([P, 4], f32)

            qkT_sb = persist.tile([P, 4, S], bf16)
            v_sb = persist.tile([P, NSC, 4, 65], bf16)
            vT_sb = persist.tile([P, 2, S], bf16)
            ones_f32 = const.tile([P, 1], f32)
            # row of ones at base partition 64, matching the denominator row
            # of psv2-staged vals (matmul requires equal base partitions)
            ones_p64 = const.tile([65, 64], f32)

            # ---- phase 1: qk projection (v-proj overlaps phase 2's first
            # ACT-paced score block) ----
            # DMA order is tuned so the first matmul can start ~1us in and
            # the PE is never starved: bias + first Wqk column block, then
            # yT for the first sb block kc-by-kc (the consumption order of
            # the first PSUM group), then the rest.  Wv/Wout only matter
            # tens of us later and go last.
            p1 = ctx_p1 = tc.alloc_tile_pool(name="p1", bufs=1)
            with tc.tile_pool(name="p1ps", bufs=4, space="PSUM") as p1ps:
                Wqk_sb = p1.tile([P, NKC, 512], bf16)
                Wqk_r = Wqk_d.rearrange("m (kc p) e -> p m kc e", p=P)
                yT_sb = p1.tile([P, NKC, S], bf16)
                yTr = yT_d.rearrange("(kc p) s -> p kc s", p=P)
                Wv_sb = p1.tile([P, NKC, 256], bf16)

                nc.sync.dma_start(Wqk_sb[:, :, 0:P], Wqk_r[:, 0])
                for kc in range(NKC):
                    nc.sync.dma_start(yT_sb[:, kc, 0:512],
                                      yTr[:, kc, 0:512])
                nc.sync.dma_start(bqk_sb[:], bqk_d)
                for m in range(1, 4):
                    nc.sync.dma_start(Wqk_sb[:, :, m * P:(m + 1) * P],
                                      Wqk_r[:, m])
                for sb in range(1, 4):
                    for kc in range(NKC):
                        sl = slice(sb * 512, (sb + 1) * 512)
                        nc.sync.dma_start(yT_sb[:, kc, sl], yTr[:, kc, sl])
                nc.sync.dma_start(
                    Wv_sb[:], Wv_d.rearrange("(kc p) e -> p kc e", p=P))
                nc.sync.dma_start(
                    Wout_sb[:], Wout_d.rearrange("(kc p) e -> p kc e", p=P))

                # HAM warmup: the PE would otherwise idle for the ~7us the
                # input DMAs take, and the first ~3.4us of real matmuls
                # would then run throttled at 1.2GHz.  Dummy matmuls on
                # (uninitialized) scratch fill the idle window so the real
                # work starts at the full 2.4GHz.
                warm_sb = p1.tile([P, P], bf16)
                nc.any.memset(warm_sb[:], 0.25)
                wps = p1ps.tile([P, P], f32, tag="warm")
                for _ in range(48):
                    nc.tensor.matmul(wps[:], warm_sb[:], warm_sb[:],
                                     start=True, stop=True)

                # const fills go after the DMA issues: nothing needs them
                # until the v projection, and putting them first would delay
                # the first input DMA (and so the first matmul) by ~2.5us
                nc.any.memset(ones_f32[:], 1.0)
                nc.any.memset(ones_p64[:], 1.0)
                nc.vector.tensor_copy(
                    v_sb[:, :, :, 64:65],
                    ones_f32.unsqueeze(1).unsqueeze(1).to_broadcast(
                        (P, NSC, 4, 1)))

                for sb in range(4):
                    for m in range(4):
                        ps = p1ps.tile([P, 512], f32, tag="proj")
                        for kc in range(NKC):
                            nc.tensor.matmul(
                                ps[:],
                                Wqk_sb[:, kc, m * P:(m + 1) * P],
                                yT_sb[:, kc, sb * 512:(sb + 1) * 512],
                                start=(kc == 0), stop=(kc == NKC - 1))
                        nc.scalar.activation(
                            qkT_sb[:, m, sb * 512:(sb + 1) * 512], ps[:],
                            AF.Identity, bias=bqk_sb[:, m:m + 1])

            # ---- phase 2: attention (per head pair, per sq block) ----
            # After each head's attn@v, the unnormalized values and the
            # reciprocal of the denominator row are staged to SBUF right away
            # (freeing the PSUM slot); the normalize tail (broadcast matmul +
            # multiply + bias + DMA into vT_sb) is deferred by two blocks so
            # the PE never waits on the ~4us DVE reciprocal.
            with (
                tc.tile_pool(name="p2e", bufs=4) as p2e,
                tc.tile_pool(name="p2s", bufs=2) as p2s,
                tc.tile_pool(name="p2ps", bufs=4, space="PSUM") as p2ps,
                tc.tile_pool(name="p2dram", bufs=4, space="DRAM") as p2dram,
                tc.tile_pool(name="p2psv", bufs=2, space="PSUM") as p2psv,
            ):
                pending = []

                def v_proj():
                    for sc in range(NSC):
                        psv = p2psv.tile([P, 256], f32, tag="vproj", bufs=2,
                                         name="psv")
                        for kc in range(NKC):
                            nc.tensor.matmul(
                                psv[:],
                                yT_sb[:, kc, sc * P:(sc + 1) * P],
                                Wv_sb[:, kc, :],
                                start=(kc == 0), stop=(kc == NKC - 1))
                        nc.vector.tensor_copy(
                            v_sb[:, sc, :, 0:64],
                            psv.rearrange("p (i d) -> p i d", i=4))

                def normalize_tail(p, sqb, sub, vals, rbs):
                    fast = rbs is None
                    sq = slice(sqb * SQB, (sqb + 1) * SQB)
                    # reciprocal of the broadcast denominators at base
                    # partition 0 (the fast approx op misbehaves at base
                    # partition 64), then multiply on the otherwise-idle
                    # GPSIMD engine.  (v-bias is folded in on the host)
                    if rbs is None:
                        # tail fast path: partition-broadcast the denominator
                        # row with a K=1 ones matmul into a freed score-ring
                        # PSUM slot instead of the (slow) DRAM bounce
                        bc = p2ps.tile([64, SQB], f32, tag="score", bufs=2,
                                       name="bc")
                        nc.tensor.matmul(bc[:], ones_p64[64:65, :],
                                         vals[64:65, :], start=True,
                                         stop=True)
                        rbs = bc
                    rbr = p2s.tile([64, SQB], f32, tag="rbs", name="rbr",
                                   bufs=8)
                    nc.vector.reciprocal_approx_fast(out=rbr[:], in_=rbs[:])
                    vtmp = p2s.tile([64, SQB], bf16, tag="vtmp", name="vtmp")
                    if fast and sub == 0:
                        # epilogue: split the two muls across engines so the
                        # final two normalize chains run in parallel
                        nc.vector.tensor_mul(vtmp[:], vals[0:64, :], rbr[:])
                    else:
                        nc.gpsimd.tensor_tensor(
                            vtmp[:], vals[0:64, :], rbr[:], Alu.mult)
                    nc.sync.dma_start(
                        vT_sb[sub * 64:(sub + 1) * 64, p, sq], vtmp[:])

                out_queue = []

                def drain_out(n):
                    # output projection, dribbled between other PE work so
                    # the vT dependencies are always a full block old and
                    # the PE never stalls on them
                    for _ in range(min(n, len(out_queue))):
                        sc, nb = out_queue.pop(0)
                        pso = p2psv.tile([P, 512], f32, tag="vproj",
                                         name="pso")
                        for kc in range(2):
                            nc.tensor.matmul(
                                pso[:],
                                vT_sb[:, kc, sc * P:(sc + 1) * P],
                                Wout_sb[:, kc, nb * 512:(nb + 1) * 512],
                                start=(kc == 0), stop=(kc == 1))
                        ost = p2s.tile([P, 512], f32, tag="ost",
                                       name="ost", bufs=3)
                        nc.scalar.copy(ost[:], pso[:])
                        nc.sync.dma_start(
                            out_d[sc * P:(sc + 1) * P,
                                  nb * 512:(nb + 1) * 512], ost[:])

                def attn_v(p, sqb, ex, fast=False):
                    """attn@v for a completed score/exp block, plus immediate
                    staging of values+reciprocal to SBUF."""
                    for sub in range(2):
                        psv2 = p2psv.tile([P, SQB], f32, tag="vt",
                                          name="psv2")
                        # reversed order: only the first matmul waits on ACT
                        # (all exps of this tile done); the rest issue
                        # back-to-back so the PE pulls LDWEIGHTS ahead and
                        # the array stays busy
                        for mk in range(NSC - 1, -1, -1):
                            nc.tensor.matmul(
                                psv2[0:65, :],
                                v_sb[:, mk, 2 * p + sub, :],
                                ex[sub][:, mk, :],
                                start=(mk == NSC - 1), stop=(mk == 0))
                        vals = p2s.tile([65, SQB], f32, tag="vals",
                                        name="vals", bufs=6)
                        if sub == 0:
                            nc.scalar.copy(vals[:], psv2[0:65, :])
                        else:
                            nc.vector.tensor_copy(vals[:], psv2[0:65, :])
                        if fast:
                            pending.append((p, sqb, sub, vals, None))
                            continue
                        rdram = p2dram.tile([1, SQB], f32, name="rdram")
                        nc.sync.dma_start(rdram[:], vals[64:65, :])
                        # issue the denominator broadcast bounce right away;
                        # by normalize time (a block later) it has landed
                        rbs = p2s.tile([64, SQB], f32, tag="rbs",
                                       name="rbs", bufs=8)
                        nc.sync.dma_start(rbs[:],
                                          rdram.to_broadcast((64, SQB)))
                        pending.append((p, sqb, sub, vals, rbs))

                def pop_pending():
                    while pending:
                        pp, psqb, psub, pvals, prb = pending.pop(0)
                        normalize_tail(pp, psqb, psub, pvals, prb)
                        if pp == 1 and psub == 1:
                            out_queue.extend(
                                (sc, nb)
                                for sc in range(psqb * 4, psqb * 4 + 4)
                                for nb in range(2))

                prev = None
                for sqb in range(NSQB):
                    for p in range(2):
                        # normalize tails first: their vT DMA chains get a
                        # ~3.6us head start on the first out-proj matmul
                        # (drained from mj 2), so the PE never waits
                        pop_pending()
                        sq = slice(sqb * SQB, (sqb + 1) * SQB)
                        exa = p2e.tile([P, NSC, SQB], bf16, tag="exp")
                        exb = p2e.tile([P, NSC, SQB], bf16, tag="exp")
                        ex = (exa, exb)
                        # two sk-chunks share one 2-bank PSUM tile; a single
                        # exp op covers both (halves drain op count).  The
                        # two units of each mj go to different engines so
                        # they drain concurrently and the score ring never
                        # waits on a same-engine exp backlog.
                        for mj in range(NSC // 2):
                            pss = [
                                p2ps.tile([P, 2, SQB], f32, tag="score",
                                          bufs=2, name="pss")
                                for _ in range(2)]
                            for half in range(2):
                                mk = 2 * mj + half
                                for sub in range(2):
                                    prt = slice(sub * 64, (sub + 1) * 64)
                                    nc.tensor.matmul(
                                        pss[sub][:, half, :],
                                        qkT_sb[prt, 2 * p + 1,
                                               mk * P:(mk + 1) * P],
                                        qkT_sb[prt, 2 * p, sq])
                            for sub in range(2):
                                exsl = ex[sub][:, 2 * mj:2 * mj + 2, :]
                                if (sub + mj) % 2 == 0:
                                    nc.vector.tensor_scalar(
                                        exsl.bitcast(i16), pss[sub][:],
                                        0.125 * SCH_A, SCH_B,
                                        Alu.mult, Alu.add)
                                else:
                                    nc.scalar.activation(
                                        exsl, pss[sub][:], AF.Exp,
                                        scale=0.125)
                            # out groups drain after the exps so an ost copy
                            # never delays an exp in the DVE queue; starting
                            # at mj 4 gives the normalize chains (recip
                            # queued behind leftover exps + bounce DMA)
                            # ~5us of head start so the out matmuls never
                            # race them
                            if mj >= 4:
                                drain_out(1)
                        # previous block's attn@v interleaves with this
                        # block's ACT-paced scores on the PE; the v
                        # projection fills the first block's gaps
                        if prev is not None:
                            attn_v(*prev)
                        else:
                            v_proj()
                        drain_out(4)
                        prev = (p, sqb, ex)
                pop_pending()
                attn_v(*prev, fast=True)
                pop_pending()
                drain_out(len(out_queue))

            ctx_p1.release()

    nc.compile()
    return nc


def _get_nc():
    if "nc" not in _CACHE:
        _CACHE["nc"] = _build_nc()
    return _CACHE["nc"]


def _host_prep(y, W_qkv, b_qkv, W_out, c):
    b = c // 4
    q = c % 4
    hs = [4 * q + i for i in range(4)]

    def Wrow(h, part):
        return W_qkv[h * 192 + part * 64: h * 192 + (part + 1) * 64]

    def brow(h, part):
        return b_qkv[h * 192 + part * 64: h * 192 + (part + 1) * 64]

    qk_rows = np.concatenate([
        Wrow(hs[0], 0), Wrow(hs[1], 0), Wrow(hs[0], 1), Wrow(hs[1], 1),
        Wrow(hs[2], 0), Wrow(hs[3], 0), Wrow(hs[2], 1), Wrow(hs[3], 1)],
        axis=0)
    bqk_flat = np.concatenate([
        brow(hs[0], 0), brow(hs[1], 0), brow(hs[0], 1), brow(hs[1], 1),
        brow(hs[2], 0), brow(hs[3], 0), brow(hs[2], 1), brow(hs[3], 1)],
        axis=0)
    import ml_dtypes

    bf = ml_dtypes.bfloat16
    # [4, 1024, 128]: m-major so each column block is one contiguous DMA
    WqkT = np.ascontiguousarray(
        qk_rows.T.reshape(1024, 4, P).transpose(1, 0, 2).astype(bf))
    bqk = np.ascontiguousarray(bqk_flat.reshape(4, P).T)     # [128, 4]
    WvT = np.ascontiguousarray(
        np.concatenate([Wrow(h, 2) for h in hs], axis=0).T.astype(bf))
    dsl = np.concatenate([np.arange(h * 64, (h + 1) * 64) for h in hs])
    WoutT = np.ascontiguousarray(W_out[:, dsl].T.astype(bf))  # [256, 1024]
    yT = np.ascontiguousarray(y[b].T.astype(bf))             # [1024, 2048]
    return {"yT": yT, "WqkT": WqkT, "bqk": bqk, "WvT": WvT,
            "WoutT": WoutT}


def _gather(results, b_qkv, W_out, b_out):
    parts = [results[c]["out"] for c in range(8)]
    # v-bias commutes through the output projection: fold it host-side
    bv_full = b_qkv.reshape(16, 3, 64)[:, 2, :].reshape(1024)
    bias = b_out + bv_full @ W_out.T
    return np.stack([
        parts[0] + parts[1] + parts[2] + parts[3] + bias,
        parts[4] + parts[5] + parts[6] + parts[7] + bias,
    ]).astype(np.float32)


def kernel(y, W_qkv, b_qkv, W_out, b_out):
    from concourse.bass_utils import run_bass_kernel_spmd

    y = np.ascontiguousarray(np.asarray(y, dtype=np.float32))
    W_qkv = np.ascontiguousarray(np.asarray(W_qkv, dtype=np.float32))
    b_qkv = np.ascontiguousarray(np.asarray(b_qkv, dtype=np.float32))
    W_out = np.ascontiguousarray(np.asarray(W_out, dtype=np.float32))
    b_out = np.asarray(b_out, dtype=np.float32)

    nc = _get_nc()
    in_maps = [_host_prep(y, W_qkv, b_qkv, W_out, c) for c in range(8)]
    res = run_bass_kernel_spmd(nc, in_maps, core_ids=list(range(8)))
    return _gather(res.results, b_qkv, W_out, b_out)



# revision 58
# speedup vs baseline: 1.0193x; 1.0193x over previous
"""Multi-head attention (B=2, S=2048, D=1024, H=16) on 8 TRN2 NeuronCores.

Sharding: core c handles batch b = c//4 and heads [4*(c%4), 4*(c%4)+4) —
tensor-parallel over heads x data-parallel over batch.  Each core computes a
partial output projection (its heads' contribution); the host sums the 4
partials per batch and adds b_out.

On-device layout (per core; bf16 matmul operands, fp32 PSUM/softmax math):
  - qk projection computed transposed: qkT [512, S], row chunks
    [q_h0|q_h1, k_h0|k_h1, q_h2|q_h3, k_h2|k_h3].  Input DMAs are ordered
    (Wqk m-block 0, yT kc-slices for the first sb block, rest) so the first
    matmul starts ~5us after the engine preamble and the PE never starves.
  - scores computed transposed: expT[sk, sq] = exp(scale * kT.T @ qT); the
    softmax denominator comes free from a ones-column appended to v in the
    attn@v matmul (out row 64 = sum over sk of expT).  No max-subtraction:
    scores*scale is ~N(0,1) so fp32 exp cannot overflow.
  - exp is split across BOTH drain engines, alternating per (sub, mj) unit:
    ACT runs native Exp; DVE runs a one-op Schraudolph exp (bf16 bits =
    round(x*128*log2e + 127*128-7.05) via tensor_scalar with an int16
    bitcast output; ~1.8% RMS on those chunks, ~0.9% end to end).  With the
    ~17us/block of exp work split 8/8, neither engine's queue ever gates
    the score-PSUM ring, which is what kept tripping HAM re-throttles
    (PE at 1.2GHz) in earlier versions.
  - attn@v: valuesT_unnorm [65, sq] = v_aug.T @ expT accumulated over sk
    chunks in reversed order (one wait, then back-to-back so LDWEIGHTS
    hides), deferred one block so the exps are provably done.
  - normalize: vals[65] staged to SBUF (copy split ACT/DVE per sub), raw
    denominator row bounced through DRAM with a stride-0 partition
    broadcast, reciprocal_approx_fast at base partition 0 (the approx op
    misbehaves at nonzero base partitions), multiply on the otherwise-idle
    GPSIMD engine.  The v-bias is linear through the output projection and
    is added on the host instead.  In the epilogue the broadcast uses a
    K=1 ones matmul into a freed score-PSUM slot instead of the DRAM
    bounce (shorter critical path), and the two muls split DVE/GPSIMD.
  - output projection out_partial = valuesT.T @ WoutT: groups are queued
    when a sq block's four normalize chains have been issued and drained
    one-per-mj from mj 4 of later blocks, so every out matmul's vT inputs
    are always a block old and the PE stream never breaks.  PSUM staging
    copies go to ACT (it has slack; DVE is the busier drain engine).
"""
import sys

sys.path.insert(0, "/opt/trn_rl_repo")

import numpy as np

B, S, D = 2, 2048, 1024
H, Hd = 16, 64
P = 128
NKC = D // P      # 8 contraction chunks for the projections
NSC = S // P      # 16 sequence chunks of 128
SQB = 512         # sq block size
NSQB = S // SQB   # 4

_CACHE = {}


def _build_nc():
    import concourse.mybir as mybir
    import concourse.tile as tile
    from concourse import bacc

    f32 = mybir.dt.float32
    f32r = mybir.dt.float32r
    bf16 = mybir.dt.bfloat16
    i16 = mybir.dt.int16
    AF = mybir.ActivationFunctionType
    Alu = mybir.AluOpType

    # Schraudolph exp on DVE: bf16 bits = round(x*scale*SCH_A + SCH_B);
    # bitcast int16 write.  RMS rel err 1.8% on the tiles it handles;
    # end-to-end (4 of 16 sk-chunks) adds ~0.2% to the final output.
    LOG2E = float(np.log2(np.e))
    SCH_A = 128.0 * LOG2E
    SCH_B = 127.0 * 128.0 - 7.05


    nc = bacc.Bacc(None, target_bir_lowering=False, debug=False)

    yT_d = nc.dram_tensor("yT", [D, S], bf16, kind="ExternalInput")[:]
    Wqk_d = nc.dram_tensor("WqkT", [4, D, P], bf16, kind="ExternalInput")[:]
    bqk_d = nc.dram_tensor("bqk", [P, 4], f32, kind="ExternalInput")[:]
    Wv_d = nc.dram_tensor("WvT", [D, 256], bf16, kind="ExternalInput")[:]
    Wout_d = nc.dram_tensor("WoutT", [256, D], bf16, kind="ExternalInput")[:]
    out_d = nc.dram_tensor("out", [S, D], f32, kind="ExternalOutput")[:]

    with tile.TileContext(nc) as tc:
        with (
            tc.tile_pool(name="const", bufs=1) as const,
            tc.tile_pool(name="persist", bufs=1) as persist,
        ):
            Wout_sb = const.tile([P, 2, D], bf16)
            bqk_sb = const.tile([P, 4], f32)

            qkT_sb = persist.tile([P, 4, S], bf16)
            v_sb = persist.tile([P, NSC, 4, 65], bf16)
            vT_sb = persist.tile([P, 2, S], bf16)
            ones_f32 = const.tile([P, 1], f32)
            # row of ones at base partition 64, matching the denominator row
            # of psv2-staged vals (matmul requires equal base partitions)
            ones_p64 = const.tile([65, 64], f32)

            # ---- phase 1: qk projection (v-proj overlaps phase 2's first
            # ACT-paced score block) ----
            # DMA order is tuned so the first matmul can start ~1us in and
            # the PE is never starved: bias + first Wqk column block, then
            # yT for the first sb block kc-by-kc (the consumption order of
            # the first PSUM group), then the rest.  Wv/Wout only matter
            # tens of us later and go last.
            p1 = ctx_p1 = tc.alloc_tile_pool(name="p1", bufs=1)
            with tc.tile_pool(name="p1ps", bufs=4, space="PSUM") as p1ps:
                Wqk_sb = p1.tile([P, NKC, 512], bf16)
                Wqk_r = Wqk_d.rearrange("m (kc p) e -> p m kc e", p=P)
                yT_sb = p1.tile([P, NKC, S], bf16)
                yTr = yT_d.rearrange("(kc p) s -> p kc s", p=P)
                Wv_sb = p1.tile([P, NKC, 256], bf16)

                nc.sync.dma_start(Wqk_sb[:, :, 0:P], Wqk_r[:, 0])
                nc.sync.dma_start(yT_sb[:, 0:4, 0:512], yTr[:, 0:4, 0:512])
                nc.sync.dma_start(yT_sb[:, 4:8, 0:512], yTr[:, 4:8, 0:512])
                nc.sync.dma_start(bqk_sb[:], bqk_d)
                for m in range(1, 4):
                    nc.sync.dma_start(Wqk_sb[:, :, m * P:(m + 1) * P],
                                      Wqk_r[:, m])
                for sb in range(1, 4):
                    sl = slice(sb * 512, (sb + 1) * 512)
                    nc.sync.dma_start(yT_sb[:, :, sl], yTr[:, :, sl])
                nc.sync.dma_start(
                    Wv_sb[:], Wv_d.rearrange("(kc p) e -> p kc e", p=P))
                nc.sync.dma_start(
                    Wout_sb[:], Wout_d.rearrange("(kc p) e -> p kc e", p=P))

                # HAM warmup: the PE would otherwise idle for the ~7us the
                # input DMAs take, and the first ~3.4us of real matmuls
                # would then run throttled at 1.2GHz.  Dummy matmuls on
                # (uninitialized) scratch fill the idle window so the real
                # work starts at the full 2.4GHz.
                warm_sb = p1.tile([P, P], bf16)
                nc.any.memset(warm_sb[:], 0.25)
                wps = p1ps.tile([P, P], f32, tag="warm")
                for _ in range(48):
                    nc.tensor.matmul(wps[:], warm_sb[:], warm_sb[:],
                                     start=True, stop=True)

                # const fills go after the DMA issues: nothing needs them
                # until the v projection, and putting them first would delay
                # the first input DMA (and so the first matmul) by ~2.5us
                nc.any.memset(ones_f32[:], 1.0)
                nc.any.memset(ones_p64[:], 1.0)
                nc.vector.tensor_copy(
                    v_sb[:, :, :, 64:65],
                    ones_f32.unsqueeze(1).unsqueeze(1).to_broadcast(
                        (P, NSC, 4, 1)))

                for sb in range(4):
                    for m in range(4):
                        ps = p1ps.tile([P, 512], f32, tag="proj")
                        for kc in range(NKC):
                            nc.tensor.matmul(
                                ps[:],
                                Wqk_sb[:, kc, m * P:(m + 1) * P],
                                yT_sb[:, kc, sb * 512:(sb + 1) * 512],
                                start=(kc == 0), stop=(kc == NKC - 1))
                        nc.scalar.activation(
                            qkT_sb[:, m, sb * 512:(sb + 1) * 512], ps[:],
                            AF.Identity, bias=bqk_sb[:, m:m + 1])

            # ---- phase 2: attention (per head pair, per sq block) ----
            # After each head's attn@v, the unnormalized values and the
            # reciprocal of the denominator row are staged to SBUF right away
            # (freeing the PSUM slot); the normalize tail (broadcast matmul +
            # multiply + bias + DMA into vT_sb) is deferred by two blocks so
            # the PE never waits on the ~4us DVE reciprocal.
            with (
                tc.tile_pool(name="p2e", bufs=4) as p2e,
                tc.tile_pool(name="p2s", bufs=2) as p2s,
                tc.tile_pool(name="p2ps", bufs=4, space="PSUM") as p2ps,
                tc.tile_pool(name="p2dram", bufs=4, space="DRAM") as p2dram,
                tc.tile_pool(name="p2psv", bufs=2, space="PSUM") as p2psv,
            ):
                pending = []

                def v_proj():
                    for sc in range(NSC):
                        psv = p2psv.tile([P, 256], f32, tag="vproj", bufs=2,
                                         name="psv")
                        for kc in range(NKC):
                            nc.tensor.matmul(
                                psv[:],
                                yT_sb[:, kc, sc * P:(sc + 1) * P],
                                Wv_sb[:, kc, :],
                                start=(kc == 0), stop=(kc == NKC - 1))
                        nc.vector.tensor_copy(
                            v_sb[:, sc, :, 0:64],
                            psv.rearrange("p (i d) -> p i d", i=4))

                def normalize_tail(p, sqb, sub, vals, rbs):
                    fast = rbs is None
                    sq = slice(sqb * SQB, (sqb + 1) * SQB)
                    # reciprocal of the broadcast denominators at base
                    # partition 0 (the fast approx op misbehaves at base
                    # partition 64), then multiply on the otherwise-idle
                    # GPSIMD engine.  (v-bias is folded in on the host)
                    if rbs is None:
                        # tail fast path: partition-broadcast the denominator
                        # row with a K=1 ones matmul into a freed score-ring
                        # PSUM slot instead of the (slow) DRAM bounce
                        bc = p2ps.tile([64, SQB], f32, tag="score", bufs=2,
                                       name="bc")
                        nc.tensor.matmul(bc[:], ones_p64[64:65, :],
                                         vals[64:65, :], start=True,
                                         stop=True)
                        rbs = bc
                    rbr = p2s.tile([64, SQB], f32, tag="rbs", name="rbr",
                                   bufs=8)
                    nc.vector.reciprocal_approx_fast(out=rbr[:], in_=rbs[:])
                    vtmp = p2s.tile([64, SQB], bf16, tag="vtmp", name="vtmp")
                    if fast and sub == 0:
                        # epilogue: split the two muls across engines so the
                        # final two normalize chains run in parallel
                        nc.vector.tensor_mul(vtmp[:], vals[0:64, :], rbr[:])
                    else:
                        nc.gpsimd.tensor_tensor(
                            vtmp[:], vals[0:64, :], rbr[:], Alu.mult)
                    nc.sync.dma_start(
                        vT_sb[sub * 64:(sub + 1) * 64, p, sq], vtmp[:])

                out_queue = []

                def drain_out(n):
                    # output projection, dribbled between other PE work so
                    # the vT dependencies are always a full block old and
                    # the PE never stalls on them
                    for _ in range(min(n, len(out_queue))):
                        sc, nb = out_queue.pop(0)
                        pso = p2psv.tile([P, 512], f32, tag="vproj",
                                         name="pso")
                        for kc in range(2):
                            nc.tensor.matmul(
                                pso[:],
                                vT_sb[:, kc, sc * P:(sc + 1) * P],
                                Wout_sb[:, kc, nb * 512:(nb + 1) * 512],
                                start=(kc == 0), stop=(kc == 1))
                        ost = p2s.tile([P, 512], f32, tag="ost",
                                       name="ost", bufs=3)
                        nc.scalar.copy(ost[:], pso[:])
                        nc.sync.dma_start(
                            out_d[sc * P:(sc + 1) * P,
                                  nb * 512:(nb + 1) * 512], ost[:])

                def attn_v(p, sqb, ex, fast=False):
                    """attn@v for a completed score/exp block, plus immediate
                    staging of values+reciprocal to SBUF."""
                    for sub in range(2):
                        psv2 = p2psv.tile([P, SQB], f32, tag="vt",
                                          name="psv2")
                        # reversed order: only the first matmul waits on ACT
                        # (all exps of this tile done); the rest issue
                        # back-to-back so the PE pulls LDWEIGHTS ahead and
                        # the array stays busy
                        for mk in range(NSC - 1, -1, -1):
                            nc.tensor.matmul(
                                psv2[0:65, :],
                                v_sb[:, mk, 2 * p + sub, :],
                                ex[sub][:, mk, :],
                                start=(mk == NSC - 1), stop=(mk == 0))
                        vals = p2s.tile([65, SQB], f32, tag="vals",
                                        name="vals", bufs=6)
                        if sub == 0:
                            nc.scalar.copy(vals[:], psv2[0:65, :])
                        else:
                            nc.vector.tensor_copy(vals[:], psv2[0:65, :])
                        if fast:
                            pending.append((p, sqb, sub, vals, None))
                            continue
                        rdram = p2dram.tile([1, SQB], f32, name="rdram")
                        nc.sync.dma_start(rdram[:], vals[64:65, :])
                        # issue the denominator broadcast bounce right away;
                        # by normalize time (a block later) it has landed
                        rbs = p2s.tile([64, SQB], f32, tag="rbs",
                                       name="rbs", bufs=8)
                        nc.sync.dma_start(rbs[:],
                                          rdram.to_broadcast((64, SQB)))
                        pending.append((p, sqb, sub, vals, rbs))

                def pop_pending():
                    while pending:
                        pp, psqb, psub, pvals, prb = pending.pop(0)
                        normalize_tail(pp, psqb, psub, pvals, prb)
                        if pp == 1 and psub == 1:
                            out_queue.extend(
                                (sc, nb)
                                for sc in range(psqb * 4, psqb * 4 + 4)
                                for nb in range(2))

                prev = None
                for sqb in range(NSQB):
                    for p in range(2):
                        # normalize tails first: their vT DMA chains get a
                        # ~3.6us head start on the first out-proj matmul
                        # (drained from mj 2), so the PE never waits
                        pop_pending()
                        sq = slice(sqb * SQB, (sqb + 1) * SQB)
                        exa = p2e.tile([P, NSC, SQB], bf16, tag="exp")
                        exb = p2e.tile([P, NSC, SQB], bf16, tag="exp")
                        ex = (exa, exb)
                        # two sk-chunks share one 2-bank PSUM tile; a single
                        # exp op covers both (halves drain op count).  The
                        # two units of each mj go to different engines so
                        # they drain concurrently and the score ring never
                        # waits on a same-engine exp backlog.
                        for mj in range(NSC // 2):
                            pss = [
                                p2ps.tile([P, 2, SQB], f32, tag="score",
                                          bufs=2, name="pss")
                                for _ in range(2)]
                            for half in range(2):
                                mk = 2 * mj + half
                                for sub in range(2):
                                    prt = slice(sub * 64, (sub + 1) * 64)
                                    nc.tensor.matmul(
                                        pss[sub][:, half, :],
                                        qkT_sb[prt, 2 * p + 1,
                                               mk * P:(mk + 1) * P],
                                        qkT_sb[prt, 2 * p, sq])
                            for sub in range(2):
                                exsl = ex[sub][:, 2 * mj:2 * mj + 2, :]
                                if (sub + mj) % 2 == 0:
                                    nc.vector.tensor_scalar(
                                        exsl.bitcast(i16), pss[sub][:],
                                        0.125 * SCH_A, SCH_B,
                                        Alu.mult, Alu.add)
                                else:
                                    nc.scalar.activation(
                                        exsl, pss[sub][:], AF.Exp,
                                        scale=0.125)
                            # out groups drain after the exps so an ost copy
                            # never delays an exp in the DVE queue; starting
                            # at mj 4 gives the normalize chains (recip
                            # queued behind leftover exps + bounce DMA)
                            # ~5us of head start so the out matmuls never
                            # race them
                            if mj >= 4:
                                drain_out(1)
                        # previous block's attn@v interleaves with this
                        # block's ACT-paced scores on the PE; the v
                        # projection fills the first block's gaps
                        if prev is not None:
                            attn_v(*prev)
                        else:
                            v_proj()
                        drain_out(4)
                        prev = (p, sqb, ex)
                pop_pending()
                attn_v(*prev, fast=True)
                pop_pending()
                drain_out(len(out_queue))

            ctx_p1.release()

    nc.compile()
    return nc


def _get_nc():
    if "nc" not in _CACHE:
        _CACHE["nc"] = _build_nc()
    return _CACHE["nc"]


def _host_prep(y, W_qkv, b_qkv, W_out, c):
    b = c // 4
    q = c % 4
    hs = [4 * q + i for i in range(4)]

    def Wrow(h, part):
        return W_qkv[h * 192 + part * 64: h * 192 + (part + 1) * 64]

    def brow(h, part):
        return b_qkv[h * 192 + part * 64: h * 192 + (part + 1) * 64]

    qk_rows = np.concatenate([
        Wrow(hs[0], 0), Wrow(hs[1], 0), Wrow(hs[0], 1), Wrow(hs[1], 1),
        Wrow(hs[2], 0), Wrow(hs[3], 0), Wrow(hs[2], 1), Wrow(hs[3], 1)],
        axis=0)
    bqk_flat = np.concatenate([
        brow(hs[0], 0), brow(hs[1], 0), brow(hs[0], 1), brow(hs[1], 1),
        brow(hs[2], 0), brow(hs[3], 0), brow(hs[2], 1), brow(hs[3], 1)],
        axis=0)
    import ml_dtypes

    bf = ml_dtypes.bfloat16
    # [4, 1024, 128]: m-major so each column block is one contiguous DMA
    WqkT = np.ascontiguousarray(
        qk_rows.T.reshape(1024, 4, P).transpose(1, 0, 2).astype(bf))
    bqk = np.ascontiguousarray(bqk_flat.reshape(4, P).T)     # [128, 4]
    WvT = np.ascontiguousarray(
        np.concatenate([Wrow(h, 2) for h in hs], axis=0).T.astype(bf))
    dsl = np.concatenate([np.arange(h * 64, (h + 1) * 64) for h in hs])
    WoutT = np.ascontiguousarray(W_out[:, dsl].T.astype(bf))  # [256, 1024]
    yT = np.ascontiguousarray(y[b].T.astype(bf))             # [1024, 2048]
    return {"yT": yT, "WqkT": WqkT, "bqk": bqk, "WvT": WvT,
            "WoutT": WoutT}


def _gather(results, b_qkv, W_out, b_out):
    parts = [results[c]["out"] for c in range(8)]
    # v-bias commutes through the output projection: fold it host-side
    bv_full = b_qkv.reshape(16, 3, 64)[:, 2, :].reshape(1024)
    bias = b_out + bv_full @ W_out.T
    return np.stack([
        parts[0] + parts[1] + parts[2] + parts[3] + bias,
        parts[4] + parts[5] + parts[6] + parts[7] + bias,
    ]).astype(np.float32)


def kernel(y, W_qkv, b_qkv, W_out, b_out):
    from concourse.bass_utils import run_bass_kernel_spmd

    y = np.ascontiguousarray(np.asarray(y, dtype=np.float32))
    W_qkv = np.ascontiguousarray(np.asarray(W_qkv, dtype=np.float32))
    b_qkv = np.ascontiguousarray(np.asarray(b_qkv, dtype=np.float32))
    W_out = np.ascontiguousarray(np.asarray(W_out, dtype=np.float32))
    b_out = np.asarray(b_out, dtype=np.float32)

    nc = _get_nc()
    in_maps = [_host_prep(y, W_qkv, b_qkv, W_out, c) for c in range(8)]
    res = run_bass_kernel_spmd(nc, in_maps, core_ids=list(range(8)))
    return _gather(res.results, b_qkv, W_out, b_out)

